# revision 8
# baseline (speedup 1.0000x reference)
"""Multi-head attention (B=2, S=2048, H=2048, NH=16) on 8 TRN2 NeuronCores.

Sharding: tensor-parallel over heads — 2 heads per core. Each core computes
q/k/v projections for its heads, per-head attention, and a partial output
projection (its heads' columns of Wo); the host sums the 8 partials.

v2: PE-dense schedule. The attention inner loop is software-pipelined
(sc(t+1)+exp(t+1) issued before av(t)) and a fill queue interleaves
qkv/oproj matmul pieces into the attention stream so the PE never idles
waiting on ScalarE exp (keeps the PE p-state at max clock). oproj PSUM
evacuation is DVE during attention fill, alternating DVE/ScalarE in the
final standalone phase (GpSimd cannot access PSUM).

Per-core dataflow (all matmuls bf16 inputs, f32 PSUM accumulation):
  - hT [H, B*S] (hidden transposed, bf16, host-prepared) streamed per batch.
  - QT/KT [hd=128, S] per (b, h): feature-major, from wT chunks (stationary)
    x hT (moving).
  - V [t, hd] token-major per b: from hT chunks (stationary) x wvT (moving).
  - scores transposed: ST[t_chunk=128, q] = (KT chunk).T @ QT -> PSUM;
    exp on ScalarE (scale=1/sqrt(hd), bias=-SHIFT) -> P^T bf16 in SBUF.
  - softmax denom: DVE pair/quad tree over P^T chunks, then ones[128,128]
    matmul (broadcasts the column-sum across all 128 partitions), reciprocal.
  - AV: out_avT[hd, q] += V[tc,hd].T @ P^T[tc] -> normalize on DVE -> aoT bf16.
  - O-proj: out[t_tile, o] += aoT[:, t_tile].T @ woT -> bf16 partial to DRAM
    (host accumulates the 8 partials in f32 and adds bo).
"""

import sys

sys.path.insert(0, "/opt/trn_rl_repo")

from contextlib import ExitStack

import ml_dtypes
import numpy as np

import concourse.bass as bass
import concourse.tile as tile
from concourse import bacc, mybir
from concourse.bass_utils import run_bass_kernel_spmd

B, S, H, NH = 2, 2048, 2048, 16
HD = H // NH          # 128
N_CORES = 8
HPC = NH // N_CORES   # heads per core = 2
HDC = HPC * HD        # head-dims per core = 256
T = B * S             # 4096 tokens
FC = H // 128         # 16 feature chunks
TC = S // 128         # 16 token tiles per batch
SHIFT = 4.0           # fixed exp shift (softmax-invariant, overflow guard)

BF16 = mybir.dt.bfloat16
F32 = mybir.dt.float32
EXP = mybir.ActivationFunctionType.Exp
COPY = mybir.ActivationFunctionType.Copy

_CACHE = {}


def build_program(out_dtype=BF16):
    nc = bacc.Bacc(
        "TRN2", target_bir_lowering=False, debug=False, num_devices=N_CORES
    )
    hT = nc.dram_tensor("hT", [H, T], BF16, kind="ExternalInput").ap()
    wqT = nc.dram_tensor("wqT", [H, HDC], BF16, kind="ExternalInput").ap()
    wkT = nc.dram_tensor("wkT", [H, HDC], BF16, kind="ExternalInput").ap()
    wvT = nc.dram_tensor("wvT", [H, HDC], BF16, kind="ExternalInput").ap()
    woT = nc.dram_tensor("woT", [HDC, H], BF16, kind="ExternalInput").ap()
    bq = nc.dram_tensor("bq", [HDC], F32, kind="ExternalInput").ap()
    bk = nc.dram_tensor("bk", [HDC], F32, kind="ExternalInput").ap()
    bv = nc.dram_tensor("bv", [1, HDC], F32, kind="ExternalInput").ap()
    out = nc.dram_tensor("out", [T, H], out_dtype, kind="ExternalOutput").ap()

    with tile.TileContext(nc) as tc:
        _kernel(tc, out, hT, wqT, wkT, wvT, woT, bq, bk, bv)
    nc.compile()
    return nc


class FillQueue:
    """Lazy queue of emit-generators; pull() emits one piece (~2-4K PE cyc)."""

    def __init__(self):
        self.units = []

    def add(self, gen):
        self.units.append(gen)

    def pull(self):
        while self.units:
            try:
                next(self.units[0])
                return True
            except StopIteration:
                self.units.pop(0)
        return False

    def drain(self):
        while self.pull():
            pass


def _kernel(tc, out, hT, wqT, wkT, wvT, woT, bq, bk, bv):
    nc = tc.nc
    scale = 1.0 / float(np.sqrt(HD))
    ctx = ExitStack()
    with ctx:
        singles = ctx.enter_context(tc.tile_pool(name="singles", bufs=1))
        persist = ctx.enter_context(tc.tile_pool(name="persist", bufs=1))
        ps_sc = ctx.enter_context(tc.tile_pool(name="ps_sc", bufs=2, space="PSUM"))
        ps_av = ctx.enter_context(tc.tile_pool(name="ps_av", bufs=1, space="PSUM"))
        ps_fill = ctx.enter_context(tc.tile_pool(name="ps_fill", bufs=1, space="PSUM"))
        ht_pool = ctx.enter_context(tc.tile_pool(name="ht", bufs=3))
        pt_pool = ctx.enter_context(tc.tile_pool(name="pt", bufs=6))
        pair_pool = ctx.enter_context(tc.tile_pool(name="pair", bufs=3))
        quad_pool = ctx.enter_context(tc.tile_pool(name="quad", bufs=5))
        den_pool = ctx.enter_context(tc.tile_pool(name="den", bufs=2))
        o_sb_pool = ctx.enter_context(tc.tile_pool(name="o_sb", bufs=4))

        # ---- constants / weights resident in SBUF ----
        ones = singles.tile([128, 128], BF16)
        nc.vector.memset(ones, 1.0)
        neg_shift = singles.tile([128, 1], F32)
        nc.vector.memset(neg_shift, -SHIFT)

        # wv first: the first matmuls (v_group of batch 0) need it.
        w_sb = {}
        for name, ap in (("v", wvT), ("q", wqT), ("k", wkT)):
            t = singles.tile([128, FC, HDC], BF16, tag=f"w{name}", name=f"w{name}")
            nc.gpsimd.dma_start(out=t, in_=ap.rearrange("(c p) m -> p c m", p=128))
            w_sb[name] = t
        woT_sb = singles.tile([128, HPC, H], BF16)
        nc.gpsimd.dma_start(out=woT_sb, in_=woT.rearrange("(h p) o -> p h o", p=128))
        bq_sb = singles.tile([128, HPC], F32)
        nc.scalar.dma_start(out=bq_sb, in_=bq.rearrange("(h p) -> p h", p=128))
        bk_sb = singles.tile([128, HPC], F32)
        nc.scalar.dma_start(out=bk_sb, in_=bk.rearrange("(h p) -> p h", p=128))
        # bv broadcast to [128, 4, 256] (stride-0 partition and group dims)
        bv4 = singles.tile([128, 4, HDC], F32)
        nc.scalar.dma_start(
            out=bv4,
            in_=bass.AP(tensor=bv.tensor, offset=bv.offset,
                        ap=[[0, 128], [0, 4], [1, HDC]]),
        )

        # persistent activations
        qt_sb = [[persist.tile([128, S], BF16, tag=f"qt{b}{h}", name=f"qt{b}{h}")
                  for h in range(HPC)] for b in range(B)]
        kt_sb = [[persist.tile([128, S], BF16, tag=f"kt{b}{h}", name=f"kt{b}{h}")
                  for h in range(HPC)] for b in range(B)]
        v_sb = [persist.tile([128, TC, HDC], BF16, tag=f"v{b}", name=f"v{b}")
                for b in range(B)]
        aoT_sb = [[persist.tile([128, S], BF16, tag=f"ao{b}{h}", name=f"ao{b}{h}")
                   for h in range(HPC)] for b in range(B)]

        hT_re = hT.rearrange("(c p) t -> p c t", p=128)

        # PSUM ring: during attention, fill units use only ps_fill (the
        # attention stream owns ps_sc/ps_av); in standalone phases rotate
        # over 4 slots to hide evacuation latency. A unit samples the ring
        # when its first piece is emitted, so switching modes affects only
        # units that start after the switch.
        class PsumRing:
            def __init__(self, slots):
                self.slots = slots
                self.i = 0

            def next(self):
                s = self.slots[self.i % len(self.slots)]
                self.i += 1
                return s

            def set_slots(self, slots):
                self.slots = slots
                self.i = 0

        RING_STANDALONE = [(ps_fill, "fill"), (ps_sc, "sc"),
                           (ps_av, "av"), (ps_sc, "sc")]
        RING_FILL = [(ps_fill, "fill")]

        ht_q = {}

        def load_half_ht(b, half):
            """DMA the two 512-token quarter tiles of hT for (b, half)."""
            qs = []
            for qx in range(2):
                t0 = b * S + half * 1024 + qx * 512
                t = ht_pool.tile([128, FC, 512], BF16, tag="ht",
                                 name=f"ht{b}{half}{qx}")
                for g in range(2):
                    nc.sync.dma_start(
                        out=t[:, 8 * g: 8 * g + 8, :],
                        in_=hT_re[:, 8 * g: 8 * g + 8, t0: t0 + 512],
                    )
                qs.append(t)
            ht_q[(b, half)] = qs

        def v_group_unit(b, half, g, ring):
            """V for 4 token sub-tiles: [128, 4, 256] psum, 16 fc each."""
            pool, tag = ring.next()
            ps = pool.tile([128, 4, HDC], F32, tag=tag,
                           name=f"vps{b}{half}{g}")
            hq = ht_q[(b, half)][g]
            for sub in range(4):
                for fc in range(FC):
                    nc.tensor.matmul(
                        ps[:, sub, :],
                        hq[:, fc, sub * 128: (sub + 1) * 128],
                        w_sb["v"][:, fc, :],
                        start=(fc == 0),
                        stop=(fc == FC - 1),
                    )
                yield  # piece boundary (~4K cycles)
            tt0 = half * 8 + g * 4
            nc.vector.tensor_add(v_sb[b][:, tt0: tt0 + 4, :], ps, bv4)

        def qk_unit(b, half, h, name, ring):
            """Q^T or K^T for one head, one 1024-token half."""
            pool, tag = ring.next()
            ps = pool.tile([128, 1024], F32, tag=tag,
                           name=f"qk{b}{half}{h}{name}")
            lhsT_all = w_sb[name]
            for fcg in range(4):  # pieces of 4 fc
                for fc in range(4 * fcg, 4 * fcg + 4):
                    lhsT = lhsT_all[:, fc, h * HD: (h + 1) * HD]
                    for n in range(2):
                        nc.tensor.matmul(
                            ps[:, n * 512: (n + 1) * 512],
                            lhsT,
                            ht_q[(b, half)][n][:, fc, :],
                            start=(fc == 0),
                            stop=(fc == FC - 1),
                        )
                if fcg < 3:
                    yield
            dst = qt_sb[b][h] if name == "q" else kt_sb[b][h]
            bias = bq_sb if name == "q" else bk_sb
            nc.vector.tensor_scalar_add(
                dst[:, half * 1024: (half + 1) * 1024], ps,
                bias[:, h: h + 1],
            )
            yield

        def evac_vec(o_tile, ps):
            nc.vector.tensor_copy(o_tile, ps)

        def evac_act(o_tile, ps):
            nc.scalar.activation(o_tile, ps, COPY)

        def oproj_unit(b, tt, half2, ring, evac):
            """One [128 tok, 1024 out-col] piece of the output projection."""
            pool, tag = ring.next()
            ps = pool.tile([128, 1024], F32, tag=tag,
                           name=f"ops{b}{tt}{half2}")
            for h in range(HPC):
                lhsT = aoT_sb[b][h][:, tt * 128: (tt + 1) * 128]
                for n in range(2):
                    o0 = half2 * 1024 + n * 512
                    nc.tensor.matmul(
                        ps[:, n * 512: (n + 1) * 512],
                        lhsT,
                        woT_sb[:, h, o0: o0 + 512],
                        start=(h == 0),
                        stop=(h == HPC - 1),
                    )
            yield
            o_tile = o_sb_pool.tile([128, 1024], out.dtype, tag="o",
                                    name=f"ot{b}{tt}{half2}")
            evac(o_tile, ps)
            row0 = b * S + tt * 128
            nc.sync.dma_start(
                out=out[row0: row0 + 128,
                        half2 * 1024: (half2 + 1) * 1024],
                in_=o_tile,
            )
            yield

        def qkv_units(b, half, ring):
            yield v_group_unit(b, half, 0, ring)
            yield qk_unit(b, half, 0, "q", ring)
            yield qk_unit(b, half, 0, "k", ring)
            yield v_group_unit(b, half, 1, ring)
            yield qk_unit(b, half, 1, "q", ring)
            yield qk_unit(b, half, 1, "k", ring)

        def oproj_units(b, tts, ring, evacs=(evac_vec,)):
            k = 0
            for tt in tts:
                for half2 in range(2):
                    yield oproj_unit(b, tt, half2, ring,
                                     evacs[k % len(evacs)])
                    k += 1

        def attention_block(b, qh, fill, fill_every=4, ep_fill=1):
            """Software-pipelined attention for 2 heads x 16 token tiles."""
            q0 = qh * 1024
            iters = [(h, t) for h in range(HPC) for t in range(TC)]

            def emit_sc(h, t):
                ps = ps_sc.tile([128, 1024], F32, tag="sc",
                                name=f"sc{b}{qh}{h}{t}")
                lhsT = kt_sb[b][h][:, t * 128: (t + 1) * 128]
                for n in range(2):
                    nc.tensor.matmul(
                        ps[:, n * 512: (n + 1) * 512],
                        lhsT,
                        qt_sb[b][h][:, q0 + n * 512: q0 + (n + 1) * 512],
                        start=True, stop=True,
                    )
                pt = pt_pool.tile([128, 1024], BF16, tag="pt",
                                  name=f"pt{b}{qh}{h}{t}")
                nc.scalar.activation(pt, ps, EXP, bias=neg_shift, scale=scale)
                return pt

            pts = {}
            pts[iters[0]] = emit_sc(*iters[0])
            av_ps = {}
            tree = {h: {"pts": [], "pairs": [], "quads": []}
                    for h in range(HPC)}
            for i, (h, t) in enumerate(iters):
                if i + 1 < len(iters):
                    pts[iters[i + 1]] = emit_sc(*iters[i + 1])
                pt = pts.pop((h, t))
                st = tree[h]
                st["pts"].append(pt)
                if t == 0:
                    av_ps[h] = ps_av.tile([128, 1024], F32, tag="av",
                                          name=f"av{b}{qh}{h}")
                for n in range(2):
                    nc.tensor.matmul(
                        av_ps[h][:, n * 512: (n + 1) * 512],
                        v_sb[b][:, t, h * HD: (h + 1) * HD],
                        pt[:, n * 512: (n + 1) * 512],
                        start=(t == 0),
                        stop=(t == TC - 1),
                    )
                # incremental DVE tree: pair at odd t, quad at t%4==3
                if t % 2 == 1:
                    pair = pair_pool.tile([128, 1024], BF16, tag="pair",
                                          name=f"pr{b}{qh}{h}{t}")
                    nc.vector.tensor_add(pair, st["pts"][-2], st["pts"][-1])
                    st["pairs"].append(pair)
                if t % 4 == 3:
                    quad = quad_pool.tile([128, 1024], BF16, tag="quad",
                                          name=f"qd{b}{qh}{h}{t}")
                    nc.vector.tensor_add(quad, st["pairs"][-2],
                                         st["pairs"][-1])
                    st["quads"].append(quad)
                if t == TC - 1:
                    den = ps_sc.tile([128, 1024], F32, tag="sc",
                                     name=f"den{b}{qh}{h}")
                    for qi in range(4):
                        for n in range(2):
                            nc.tensor.matmul(
                                den[:, n * 512: (n + 1) * 512],
                                ones,
                                st["quads"][qi][:, n * 512: (n + 1) * 512],
                                start=(qi == 0),
                                stop=(qi == 3),
                            )
                    recip = den_pool.tile([128, 1024], F32, tag="recip",
                                          name=f"r{b}{qh}{h}")
                    nc.vector.reciprocal_approx_fast(recip, den)
                    nc.vector.tensor_mul(
                        aoT_sb[b][h][:, q0: q0 + 1024], av_ps[h], recip)
                    for _ in range(ep_fill):
                        fill.pull()
                elif i % fill_every == fill_every - 1:
                    fill.pull()

        # ---------------- schedule ----------------
        fill = FillQueue()
        ring = PsumRing(RING_STANDALONE)

        # S0: qkv for batch 0, standalone (PE-dense, 4-slot psum ring)
        load_half_ht(0, 0)
        load_half_ht(0, 1)
        s0 = FillQueue()
        for half in range(2):
            for u in qkv_units(0, half, ring):
                s0.add(u)
        s0.drain()

        # S1: attention(0,0) with qkv(1,0) fill
        ring.set_slots(RING_FILL)
        load_half_ht(1, 0)
        for u in qkv_units(1, 0, ring):
            fill.add(u)
        attention_block(0, 0, fill)

        # S2: attention(0,1) with qkv(1,1) fill
        load_half_ht(1, 1)
        for u in qkv_units(1, 1, ring):
            fill.add(u)
        attention_block(0, 1, fill)

        # S2.5: drain remaining qkv(1,*) with the standalone ring
        ring.set_slots(RING_STANDALONE)
        fill.drain()

        # S3: attention(1,0) with oproj(0, tt 0..7) fill
        ring.set_slots(RING_FILL)
        for u in oproj_units(0, range(0, 8), ring):
            fill.add(u)
        attention_block(1, 0, fill)

        # S4: attention(1,1) with oproj(0, 8..15) + oproj(1, 0..7) fill
        for u in oproj_units(0, range(8, TC), ring):
            fill.add(u)
        for u in oproj_units(1, range(0, 8), ring):
            fill.add(u)
        attention_block(1, 1, fill)

        # S5: drain remaining fill + oproj(1, 8..15), standalone ring.
        # ScalarE is idle here (no more exps), so alternate DVE/ScalarE
        # for the PSUM evacuations to keep the phase PE-bound.
        ring.set_slots(RING_STANDALONE)
        for u in oproj_units(1, range(8, TC), ring, evacs=(evac_vec, evac_act)):
            fill.add(u)
        fill.drain()


def kernel(hidden_state, Wq, bq, Wk, bk, Wv, bv, Wo, bo):
    bf16 = ml_dtypes.bfloat16
    h2 = np.asarray(hidden_state, dtype=np.float32).reshape(T, H)
    hT = np.ascontiguousarray(h2.T).astype(bf16)

    in_maps = []
    for c in range(N_CORES):
        r0 = c * HDC
        in_maps.append({
            "hT": hT,
            "wqT": np.ascontiguousarray(
                np.asarray(Wq, np.float32)[r0: r0 + HDC, :].T).astype(bf16),
            "wkT": np.ascontiguousarray(
                np.asarray(Wk, np.float32)[r0: r0 + HDC, :].T).astype(bf16),
            "wvT": np.ascontiguousarray(
                np.asarray(Wv, np.float32)[r0: r0 + HDC, :].T).astype(bf16),
            "woT": np.ascontiguousarray(
                np.asarray(Wo, np.float32)[:, r0: r0 + HDC].T).astype(bf16),
            "bq": np.asarray(bq, np.float32)[r0: r0 + HDC].copy(),
            "bk": np.asarray(bk, np.float32)[r0: r0 + HDC].copy(),
            "bv": np.asarray(bv, np.float32)[r0: r0 + HDC].reshape(1, HDC).copy(),
        })

    if "nc" not in _CACHE:
        _CACHE["nc"] = build_program()
    nc = _CACHE["nc"]
    _CACHE["in_maps"] = in_maps

    res = run_bass_kernel_spmd(nc, in_maps, core_ids=list(range(N_CORES)))
    total = np.zeros((T, H), np.float32)
    for r in res.results:
        total += np.asarray(r["out"]).astype(np.float32)
    total += np.asarray(bo, np.float32)[None, :]
    return total.reshape(B, S, H)


# revision 10
# speedup vs baseline: 1.0299x; 1.0299x over previous
"""Multi-head attention (B=2, S=2048, H=2048, NH=16) on 8 TRN2 NeuronCores.

Sharding: tensor-parallel over heads — 2 heads per core. Each core computes
q/k/v projections for its heads, per-head attention, and a partial output
projection (its heads' columns of Wo); the host sums the 8 partials.

v2: PE-dense schedule. The attention inner loop is software-pipelined
(sc(t+1)+exp(t+1) issued before av(t)) and a fill queue interleaves
qkv/oproj matmul pieces into the attention stream so the PE never idles
waiting on ScalarE exp (keeps the PE p-state at max clock). oproj PSUM
evacuation is DVE during attention fill, alternating DVE/ScalarE in the
final standalone phase (GpSimd cannot access PSUM).

Per-core dataflow (all matmuls bf16 inputs, f32 PSUM accumulation):
  - hT [H, B*S] (hidden transposed, bf16, host-prepared) streamed per batch.
  - QT/KT [hd=128, S] per (b, h): feature-major, from wT chunks (stationary)
    x hT (moving).
  - V [t, hd] token-major per b: from hT chunks (stationary) x wvT (moving).
  - scores transposed: ST[t_chunk=128, q] = (KT chunk).T @ QT -> PSUM;
    exp on ScalarE (scale=1/sqrt(hd), bias=-SHIFT) -> P^T bf16 in SBUF.
  - softmax denom: DVE pair/quad tree over P^T chunks, then ones[128,128]
    matmul (broadcasts the column-sum across all 128 partitions), reciprocal.
  - AV: out_avT[hd, q] += V[tc,hd].T @ P^T[tc] -> normalize on DVE -> aoT bf16.
  - O-proj: out[t_tile, o] += aoT[:, t_tile].T @ woT -> bf16 partial to DRAM
    (host accumulates the 8 partials in f32 and adds bo).
"""

import sys

sys.path.insert(0, "/opt/trn_rl_repo")

from contextlib import ExitStack

import ml_dtypes
import numpy as np

import concourse.bass as bass
import concourse.tile as tile
from concourse import bacc, mybir
from concourse.bass_utils import run_bass_kernel_spmd

B, S, H, NH = 2, 2048, 2048, 16
HD = H // NH          # 128
N_CORES = 8
HPC = NH // N_CORES   # heads per core = 2
HDC = HPC * HD        # head-dims per core = 256
T = B * S             # 4096 tokens
FC = H // 128         # 16 feature chunks
TC = S // 128         # 16 token tiles per batch
SHIFT = 4.0           # fixed exp shift (softmax-invariant, overflow guard)

BF16 = mybir.dt.bfloat16
F32 = mybir.dt.float32
EXP = mybir.ActivationFunctionType.Exp
COPY = mybir.ActivationFunctionType.Copy

_CACHE = {}


def build_program(out_dtype=BF16):
    nc = bacc.Bacc(
        "TRN2", target_bir_lowering=False, debug=False, num_devices=N_CORES
    )
    hT = nc.dram_tensor("hT", [H, T], BF16, kind="ExternalInput").ap()
    wqT = nc.dram_tensor("wqT", [H, HDC], BF16, kind="ExternalInput").ap()
    wkT = nc.dram_tensor("wkT", [H, HDC], BF16, kind="ExternalInput").ap()
    wvT = nc.dram_tensor("wvT", [H, HDC], BF16, kind="ExternalInput").ap()
    woT = nc.dram_tensor("woT", [HDC, H], BF16, kind="ExternalInput").ap()
    bq = nc.dram_tensor("bq", [HDC], F32, kind="ExternalInput").ap()
    bk = nc.dram_tensor("bk", [HDC], F32, kind="ExternalInput").ap()
    bv = nc.dram_tensor("bv", [1, HDC], F32, kind="ExternalInput").ap()
    out = nc.dram_tensor("out", [T, H], out_dtype, kind="ExternalOutput").ap()

    with tile.TileContext(nc) as tc:
        _kernel(tc, out, hT, wqT, wkT, wvT, woT, bq, bk, bv)
    nc.compile()
    return nc


class FillQueue:
    """Lazy queue of emit-generators; pull() emits one piece (~2-4K PE cyc)."""

    def __init__(self):
        self.units = []

    def add(self, gen):
        self.units.append(gen)

    def pull(self):
        while self.units:
            try:
                next(self.units[0])
                return True
            except StopIteration:
                self.units.pop(0)
        return False

    def drain(self):
        while self.pull():
            pass


def _kernel(tc, out, hT, wqT, wkT, wvT, woT, bq, bk, bv):
    nc = tc.nc
    scale = 1.0 / float(np.sqrt(HD))
    ctx = ExitStack()
    with ctx:
        singles = ctx.enter_context(tc.tile_pool(name="singles", bufs=1))
        persist = ctx.enter_context(tc.tile_pool(name="persist", bufs=1))
        ps_sc = ctx.enter_context(tc.tile_pool(name="ps_sc", bufs=2, space="PSUM"))
        ps_av = ctx.enter_context(tc.tile_pool(name="ps_av", bufs=1, space="PSUM"))
        ps_fill = ctx.enter_context(tc.tile_pool(name="ps_fill", bufs=1, space="PSUM"))
        ht_pool = ctx.enter_context(tc.tile_pool(name="ht", bufs=3))
        pt_pool = ctx.enter_context(tc.tile_pool(name="pt", bufs=6))
        pair_pool = ctx.enter_context(tc.tile_pool(name="pair", bufs=3))
        quad_pool = ctx.enter_context(tc.tile_pool(name="quad", bufs=5))
        den_pool = ctx.enter_context(tc.tile_pool(name="den", bufs=2))
        o_sb_pool = ctx.enter_context(tc.tile_pool(name="o_sb", bufs=4))

        # ---- constants / weights resident in SBUF ----
        ones = singles.tile([128, 128], BF16)
        nc.vector.memset(ones, 1.0)
        neg_shift = singles.tile([128, 1], F32)
        nc.vector.memset(neg_shift, -SHIFT)

        # Parallel weight loads: one DMA queue per tensor so wq/wk do not
        # serialize behind wv (each ~1MB strided load takes ~3-8us).
        w_sb = {}
        for name, ap, eng in (("v", wvT, nc.gpsimd), ("q", wqT, nc.scalar),
                              ("k", wkT, nc.gpsimd)):
            t = singles.tile([128, FC, HDC], BF16, tag=f"w{name}", name=f"w{name}")
            eng.dma_start(out=t, in_=ap.rearrange("(c p) m -> p c m", p=128))
            w_sb[name] = t
        woT_sb = singles.tile([128, HPC, H], BF16)
        nc.gpsimd.dma_start(out=woT_sb, in_=woT.rearrange("(h p) o -> p h o", p=128))
        bq_sb = singles.tile([128, HPC], F32)
        nc.scalar.dma_start(out=bq_sb, in_=bq.rearrange("(h p) -> p h", p=128))
        bk_sb = singles.tile([128, HPC], F32)
        nc.scalar.dma_start(out=bk_sb, in_=bk.rearrange("(h p) -> p h", p=128))
        # bv broadcast to [128, 4, 256] (stride-0 partition and group dims)
        bv4 = singles.tile([128, 4, HDC], F32)
        nc.scalar.dma_start(
            out=bv4,
            in_=bass.AP(tensor=bv.tensor, offset=bv.offset,
                        ap=[[0, 128], [0, 4], [1, HDC]]),
        )

        # persistent activations
        qt_sb = [[persist.tile([128, S], BF16, tag=f"qt{b}{h}", name=f"qt{b}{h}")
                  for h in range(HPC)] for b in range(B)]
        kt_sb = [[persist.tile([128, S], BF16, tag=f"kt{b}{h}", name=f"kt{b}{h}")
                  for h in range(HPC)] for b in range(B)]
        v_sb = [persist.tile([128, TC, HDC], BF16, tag=f"v{b}", name=f"v{b}")
                for b in range(B)]
        aoT_sb = [[persist.tile([128, S], BF16, tag=f"ao{b}{h}", name=f"ao{b}{h}")
                   for h in range(HPC)] for b in range(B)]

        hT_re = hT.rearrange("(c p) t -> p c t", p=128)

        # PSUM ring: during attention, fill units use only ps_fill (the
        # attention stream owns ps_sc/ps_av); in standalone phases rotate
        # over 4 slots to hide evacuation latency. A unit samples the ring
        # when its first piece is emitted, so switching modes affects only
        # units that start after the switch.
        class PsumRing:
            def __init__(self, slots):
                self.slots = slots
                self.i = 0

            def next(self):
                s = self.slots[self.i % len(self.slots)]
                self.i += 1
                return s

            def set_slots(self, slots):
                self.slots = slots
                self.i = 0

        RING_STANDALONE = [(ps_fill, "fill"), (ps_sc, "sc"),
                           (ps_av, "av"), (ps_sc, "sc")]
        RING_FILL = [(ps_fill, "fill")]

        ht_q = {}

        def load_half_ht(b, half):
            """DMA the two 512-token quarter tiles of hT for (b, half)."""
            qs = []
            for qx in range(2):
                t0 = b * S + half * 1024 + qx * 512
                t = ht_pool.tile([128, FC, 512], BF16, tag="ht",
                                 name=f"ht{b}{half}{qx}")
                for g in range(2):
                    nc.sync.dma_start(
                        out=t[:, 8 * g: 8 * g + 8, :],
                        in_=hT_re[:, 8 * g: 8 * g + 8, t0: t0 + 512],
                    )
                qs.append(t)
            ht_q[(b, half)] = qs

        def v_group_unit(b, half, g, ring):
            """V for 4 token sub-tiles: [128, 4, 256] psum, 16 fc each."""
            pool, tag = ring.next()
            ps = pool.tile([128, 4, HDC], F32, tag=tag,
                           name=f"vps{b}{half}{g}")
            hq = ht_q[(b, half)][g]
            for sub in range(4):
                for fc in range(FC):
                    nc.tensor.matmul(
                        ps[:, sub, :],
                        hq[:, fc, sub * 128: (sub + 1) * 128],
                        w_sb["v"][:, fc, :],
                        start=(fc == 0),
                        stop=(fc == FC - 1),
                    )
                yield  # piece boundary (~4K cycles)
            tt0 = half * 8 + g * 4
            nc.vector.tensor_add(v_sb[b][:, tt0: tt0 + 4, :], ps, bv4)

        def qk_unit(b, half, h, name, ring):
            """Q^T or K^T for one head, one 1024-token half."""
            pool, tag = ring.next()
            ps = pool.tile([128, 1024], F32, tag=tag,
                           name=f"qk{b}{half}{h}{name}")
            lhsT_all = w_sb[name]
            for fcg in range(4):  # pieces of 4 fc
                for fc in range(4 * fcg, 4 * fcg + 4):
                    lhsT = lhsT_all[:, fc, h * HD: (h + 1) * HD]
                    for n in range(2):
                        nc.tensor.matmul(
                            ps[:, n * 512: (n + 1) * 512],
                            lhsT,
                            ht_q[(b, half)][n][:, fc, :],
                            start=(fc == 0),
                            stop=(fc == FC - 1),
                        )
                if fcg < 3:
                    yield
            dst = qt_sb[b][h] if name == "q" else kt_sb[b][h]
            bias = bq_sb if name == "q" else bk_sb
            nc.vector.tensor_scalar_add(
                dst[:, half * 1024: (half + 1) * 1024], ps,
                bias[:, h: h + 1],
            )
            yield

        def evac_vec(o_tile, ps):
            nc.vector.tensor_copy(o_tile, ps)

        def evac_act(o_tile, ps):
            nc.scalar.activation(o_tile, ps, COPY)

        def oproj_unit(b, tt, half2, ring, evac):
            """One [128 tok, 1024 out-col] piece of the output projection."""
            pool, tag = ring.next()
            ps = pool.tile([128, 1024], F32, tag=tag,
                           name=f"ops{b}{tt}{half2}")
            for h in range(HPC):
                lhsT = aoT_sb[b][h][:, tt * 128: (tt + 1) * 128]
                for n in range(2):
                    o0 = half2 * 1024 + n * 512
                    nc.tensor.matmul(
                        ps[:, n * 512: (n + 1) * 512],
                        lhsT,
                        woT_sb[:, h, o0: o0 + 512],
                        start=(h == 0),
                        stop=(h == HPC - 1),
                    )
            yield
            o_tile = o_sb_pool.tile([128, 1024], out.dtype, tag="o",
                                    name=f"ot{b}{tt}{half2}")
            evac(o_tile, ps)
            row0 = b * S + tt * 128
            nc.sync.dma_start(
                out=out[row0: row0 + 128,
                        half2 * 1024: (half2 + 1) * 1024],
                in_=o_tile,
            )
            yield

        def qkv_units(b, half, ring, v_first=False):
            if v_first:
                yield v_group_unit(b, half, 0, ring)
                yield v_group_unit(b, half, 1, ring)
                yield qk_unit(b, half, 0, "q", ring)
                yield qk_unit(b, half, 0, "k", ring)
                yield qk_unit(b, half, 1, "q", ring)
                yield qk_unit(b, half, 1, "k", ring)
            else:
                yield v_group_unit(b, half, 0, ring)
                yield qk_unit(b, half, 0, "q", ring)
                yield qk_unit(b, half, 0, "k", ring)
                yield v_group_unit(b, half, 1, ring)
                yield qk_unit(b, half, 1, "q", ring)
                yield qk_unit(b, half, 1, "k", ring)

        def oproj_units(b, tts, ring, evacs=(evac_vec,)):
            k = 0
            for tt in tts:
                for half2 in range(2):
                    yield oproj_unit(b, tt, half2, ring,
                                     evacs[k % len(evacs)])
                    k += 1

        def attention_block(b, qh, fill, fill_every=1, ep_fill=2):
            """Software-pipelined attention for 2 heads x 16 token tiles."""
            q0 = qh * 1024
            iters = [(h, t) for h in range(HPC) for t in range(TC)]

            def emit_sc(h, t):
                ps = ps_sc.tile([128, 1024], F32, tag="sc",
                                name=f"sc{b}{qh}{h}{t}")
                lhsT = kt_sb[b][h][:, t * 128: (t + 1) * 128]
                for n in range(2):
                    nc.tensor.matmul(
                        ps[:, n * 512: (n + 1) * 512],
                        lhsT,
                        qt_sb[b][h][:, q0 + n * 512: q0 + (n + 1) * 512],
                        start=True, stop=True,
                    )
                pt = pt_pool.tile([128, 1024], BF16, tag="pt",
                                  name=f"pt{b}{qh}{h}{t}")
                nc.scalar.activation(pt, ps, EXP, bias=neg_shift, scale=scale)
                return pt

            pts = {}
            pts[iters[0]] = emit_sc(*iters[0])
            av_ps = {}
            tree = {h: {"pts": [], "pairs": [], "quads": []}
                    for h in range(HPC)}
            for i, (h, t) in enumerate(iters):
                if i + 1 < len(iters):
                    pts[iters[i + 1]] = emit_sc(*iters[i + 1])
                pt = pts.pop((h, t))
                st = tree[h]
                st["pts"].append(pt)
                if t == 0:
                    av_ps[h] = ps_av.tile([128, 1024], F32, tag="av",
                                          name=f"av{b}{qh}{h}")
                for n in range(2):
                    nc.tensor.matmul(
                        av_ps[h][:, n * 512: (n + 1) * 512],
                        v_sb[b][:, t, h * HD: (h + 1) * HD],
                        pt[:, n * 512: (n + 1) * 512],
                        start=(t == 0),
                        stop=(t == TC - 1),
                    )
                # incremental DVE tree: pair at odd t, quad at t%4==3
                if t % 2 == 1:
                    pair = pair_pool.tile([128, 1024], BF16, tag="pair",
                                          name=f"pr{b}{qh}{h}{t}")
                    nc.vector.tensor_add(pair, st["pts"][-2], st["pts"][-1])
                    st["pairs"].append(pair)
                if t % 4 == 3:
                    quad = quad_pool.tile([128, 1024], BF16, tag="quad",
                                          name=f"qd{b}{qh}{h}{t}")
                    nc.vector.tensor_add(quad, st["pairs"][-2],
                                         st["pairs"][-1])
                    st["quads"].append(quad)
                if t == TC - 1:
                    den = ps_sc.tile([128, 1024], F32, tag="sc",
                                     name=f"den{b}{qh}{h}")
                    for qi in range(4):
                        for n in range(2):
                            nc.tensor.matmul(
                                den[:, n * 512: (n + 1) * 512],
                                ones,
                                st["quads"][qi][:, n * 512: (n + 1) * 512],
                                start=(qi == 0),
                                stop=(qi == 3),
                            )
                    recip = den_pool.tile([128, 1024], F32, tag="recip",
                                          name=f"r{b}{qh}{h}")
                    nc.vector.reciprocal_approx_fast(recip, den)
                    nc.vector.tensor_mul(
                        aoT_sb[b][h][:, q0: q0 + 1024], av_ps[h], recip)
                    for _ in range(ep_fill):
                        fill.pull()
                elif i % fill_every == fill_every - 1:
                    fill.pull()

        # ---------------- schedule ----------------
        fill = FillQueue()
        ring = PsumRing(RING_STANDALONE)

        # S0: qkv for batch 0, standalone (PE-dense, 4-slot psum ring)
        load_half_ht(0, 0)
        load_half_ht(0, 1)
        s0 = FillQueue()
        for half in range(2):
            for u in qkv_units(0, half, ring, v_first=(half == 0)):
                s0.add(u)
        s0.drain()

        # S1: attention(0,0) with qkv(1,0) fill
        ring.set_slots(RING_FILL)
        load_half_ht(1, 0)
        for u in qkv_units(1, 0, ring):
            fill.add(u)
        attention_block(0, 0, fill)

        # S2: attention(0,1) with qkv(1,1) fill
        load_half_ht(1, 1)
        for u in qkv_units(1, 1, ring):
            fill.add(u)
        attention_block(0, 1, fill)

        # S2.5: drain remaining qkv(1,*) with the standalone ring
        ring.set_slots(RING_STANDALONE)
        fill.drain()

        # S3: attention(1,0) with oproj(0, tt 0..7) fill
        ring.set_slots(RING_FILL)
        for u in oproj_units(0, range(0, 8), ring):
            fill.add(u)
        attention_block(1, 0, fill)

        # S4: attention(1,1) with oproj(0, 8..15) + oproj(1, 0..7) fill
        for u in oproj_units(0, range(8, TC), ring):
            fill.add(u)
        for u in oproj_units(1, range(0, 8), ring):
            fill.add(u)
        attention_block(1, 1, fill)

        # S5: drain remaining fill + oproj(1, 8..15), standalone ring.
        # ScalarE is idle here (no more exps), so alternate DVE/ScalarE
        # for the PSUM evacuations to keep the phase PE-bound.
        ring.set_slots(RING_STANDALONE)
        for u in oproj_units(1, range(8, TC), ring, evacs=(evac_vec, evac_act)):
            fill.add(u)
        fill.drain()


def kernel(hidden_state, Wq, bq, Wk, bk, Wv, bv, Wo, bo):
    bf16 = ml_dtypes.bfloat16
    h2 = np.asarray(hidden_state, dtype=np.float32).reshape(T, H)
    hT = np.ascontiguousarray(h2.T).astype(bf16)

    in_maps = []
    for c in range(N_CORES):
        r0 = c * HDC
        in_maps.append({
            "hT": hT,
            "wqT": np.ascontiguousarray(
                np.asarray(Wq, np.float32)[r0: r0 + HDC, :].T).astype(bf16),
            "wkT": np.ascontiguousarray(
                np.asarray(Wk, np.float32)[r0: r0 + HDC, :].T).astype(bf16),
            "wvT": np.ascontiguousarray(
                np.asarray(Wv, np.float32)[r0: r0 + HDC, :].T).astype(bf16),
            "woT": np.ascontiguousarray(
                np.asarray(Wo, np.float32)[:, r0: r0 + HDC].T).astype(bf16),
            "bq": np.asarray(bq, np.float32)[r0: r0 + HDC].copy(),
            "bk": np.asarray(bk, np.float32)[r0: r0 + HDC].copy(),
            "bv": np.asarray(bv, np.float32)[r0: r0 + HDC].reshape(1, HDC).copy(),
        })

    if "nc" not in _CACHE:
        _CACHE["nc"] = build_program()
    nc = _CACHE["nc"]
    _CACHE["in_maps"] = in_maps

    res = run_bass_kernel_spmd(nc, in_maps, core_ids=list(range(N_CORES)))
    total = np.zeros((T, H), np.float32)
    for r in res.results:
        total += np.asarray(r["out"]).astype(np.float32)
    total += np.asarray(bo, np.float32)[None, :]
    return total.reshape(B, S, H)
